# revision 9
# baseline (speedup 1.0000x reference)
"""Trainium2 Bass kernel for ComplexAttention (v2).

Math (per (b,t) pair):
    cur2 = [cur_r, cur_i]                       # [2D]
    Q    = cur2 @ qW + qb                       # [D]
    K_s  = H_s @ kW + kb ; V_s = H_s @ vW + vb  # H = [hist_r, hist_i]  [S, 2D]
    sc_s = (Q . K_s) * scale * conf
    w    = softmax(sc) ; ctx = sum_s w_s V_s
    out  = cur + 0.1 * ctx (complex)

Rewrites (exact):
    Q . K_s = (cur2 @ (qW kW^T) + qb kW^T) . H_s  (+ const-over-s term, dropped)
    ctx = (sum_s w_s H_s) @ vW + vb               (since sum_s w_s = 1)

Per-core structure (512 pairs/core, data-parallel over 8 cores):
  phase A (batch of 128 pairs): Qk = cur2 @ Wqk + bqk  (bf16, overlaps H DMA)
  per quad (16 pairs = 4 groups of 4, one 2MB H tile [128=(j,s), 4, 2D] f32):
    per group: qkr = replicate Qk rows across 32 slots (bf16 row-tiled matmul)
               scores = fused DVE scalar_tensor_tensor:
                   sink = (H * conf*scale) * qkr, accum_out = scores col
    softmax: exp (ACT) -> dn replication matmul (block-ones) -> recip, wn=exp/dn
    per group: wd = m4 * wn[:,g] (gpsimd), hb = wd^T @ H (M=4 matmul),
               hsb = copy hb -> f16 (ACT), gather into hbar (gpsimd DMA)
  per batch: transpose hbar (PE), ctx = hbarT^T @ vW + vb, out = cur + 0.1*ctx
"""

import os
import sys

import numpy as np

os.environ.setdefault("MYCRO_LOCAL_CACHE", "1")

try:
    import concourse.bass as bass
except ImportError:  # pragma: no cover
    sys.path.insert(0, "/opt/trn_rl_repo")
    import concourse.bass as bass

import concourse.mybir as mybir
import concourse.tile as tile
from concourse import bacc
from concourse.bass_utils import run_bass_kernel_spmd

F32 = mybir.dt.float32
F32R = mybir.dt.float32r
F16 = mybir.dt.float16
BF16 = mybir.dt.bfloat16
AX = mybir.AluOpType
AF = mybir.ActivationFunctionType

B, T, S, D = 4, 1024, 32, 512
D2 = 2 * D  # 1024, concat(real, imag) feature dim
E = 2 * D   # 1024, history feature dim
N_CORES = 8
PAIRS = B * T
SCALE = float(D) ** -0.5


def build(ppc: int) -> bass.Bass:
    """Build the per-core SPMD program for `ppc` pairs per core."""
    assert ppc % 128 == 0
    nb = ppc // 128      # batches of 128 pairs
    nq = ppc // 16       # quads of 16 pairs (4 groups)
    ng = ppc // 4        # groups of 4 pairs

    nc = bacc.Bacc("TRN2", target_bir_lowering=False)

    hist_r = nc.declare_dram_parameter("hist_real", [ppc, S, D], F32, isOutput=False)
    hist_i = nc.declare_dram_parameter("hist_imag", [ppc, S, D], F32, isOutput=False)
    cur_r = nc.declare_dram_parameter("cur_r", [ppc, D], F32, isOutput=False)
    cur_i = nc.declare_dram_parameter("cur_i", [ppc, D], F32, isOutput=False)
    cur2t = nc.declare_dram_parameter("cur2t", [D2, ppc], BF16, isOutput=False)
    wqk = nc.declare_dram_parameter("wqk", [D2, E], BF16, isOutput=False)
    bqk = nc.declare_dram_parameter("bqk", [1, E], BF16, isOutput=False)
    vw = nc.declare_dram_parameter("vw", [E, E], F16, isOutput=False)
    conf_rep = nc.declare_dram_parameter("conf_rep", [128, ng], F32, isOutput=False)
    mask32 = nc.declare_dram_parameter("mask32", [128, 32, 128], BF16, isOutput=False)
    m4 = nc.declare_dram_parameter("m4", [128, 4], F32, isOutput=False)
    ones1b = nc.declare_dram_parameter("ones1b", [1, 128], BF16, isOutput=False)
    ident = nc.declare_dram_parameter("ident", [128, 128], F16, isOutput=False)
    out = nc.declare_dram_parameter("out", [ppc, D, 2], F32, isOutput=True)

    from contextlib import ExitStack

    with tile.TileContext(nc) as tc, ExitStack() as es:
        ec = es.enter_context
        cpool = ec(tc.tile_pool(name="const", bufs=1))
        wpool = ec(tc.tile_pool(name="bigw", bufs=1))
        curpool = ec(tc.tile_pool(name="cur", bufs=2))
        qkpool = ec(tc.tile_pool(name="qk", bufs=4))
        hpool = ec(tc.tile_pool(name="h", bufs=4))
        sinkpool = ec(tc.tile_pool(name="sink", bufs=2))
        smpool = ec(tc.tile_pool(name="sm", bufs=3))
        wdpool = ec(tc.tile_pool(name="wd", bufs=4))
        hsbpool = ec(tc.tile_pool(name="hsb", bufs=3))
        hbpool = ec(tc.tile_pool(name="hbarb", bufs=2))
        htpool = ec(tc.tile_pool(name="hbarT", bufs=2))
        outpool = ec(tc.tile_pool(name="outp", bufs=2))
        ps_qkr = ec(tc.tile_pool(name="ps_qkr", bufs=2, space="PSUM"))
        ps_hb = ec(tc.tile_pool(name="ps_hb", bufs=1, space="PSUM"))
        ps_misc = ec(tc.tile_pool(name="ps_misc", bufs=2, space="PSUM"))
        del es

        # ---- constants / weights resident in SBUF ----
        m4_t = cpool.tile([128, 4], F32)
        nc.sync.dma_start(out=m4_t[:], in_=m4[:])
        ones1b_t = cpool.tile([1, 128], BF16)
        nc.sync.dma_start(out=ones1b_t[:], in_=ones1b[:])
        id_t = cpool.tile([128, 128], F16)
        nc.sync.dma_start(out=id_t[:], in_=ident[:])
        bqk_t = cpool.tile([1, E], BF16)
        nc.sync.dma_start(out=bqk_t[:], in_=bqk[:])
        cr_t = cpool.tile([128, ng], F32)
        nc.sync.dma_start(out=cr_t[:], in_=conf_rep[:])
        mask_t = cpool.tile([128, 32, 128], BF16)
        nc.sync.dma_start(out=mask_t[:], in_=mask32[:])

        c2t_t = wpool.tile([128, 8, ppc], BF16, tag="c2t")
        nc.sync.dma_start(
            out=c2t_t[:],
            in_=cur2t[:].rearrange("(k p) n -> p k n", p=128),
        )
        wqk_t = wpool.tile([128, 8, E], BF16, tag="wqk")
        nc.sync.dma_start(
            out=wqk_t[:],
            in_=wqk[:].rearrange("(k p) e -> p k e", p=128),
        )
        vw_t = wpool.tile([128, 8, E], F16, tag="vw")
        nc.sync.dma_start(
            out=vw_t[:],
            in_=vw[:].rearrange("(k p) e -> p k e", p=128),
        )

        # ---- phase A: Qk = cur2 @ Wqk + bqk, kept as bf16 per batch ----
        qks = []
        for b in range(nb):
            qk_t = qkpool.tile([128, E], BF16, tag="qk")
            for h in range(2):
                ps = ps_misc.tile([128, 512], F32, tag="misc")
                for k in range(8):
                    nc.tensor.matmul(
                        ps[:],
                        lhsT=c2t_t[:, k, 128 * b : 128 * (b + 1)],
                        rhs=wqk_t[:, k, 512 * h : 512 * (h + 1)],
                        start=(k == 0),
                        stop=False,
                    )
                nc.tensor.matmul(
                    ps[:],
                    lhsT=ones1b_t[:],
                    rhs=bqk_t[:, 512 * h : 512 * (h + 1)],
                    start=False,
                    stop=True,
                )
                nc.scalar.activation(
                    qk_t[:, 512 * h : 512 * (h + 1)], ps[:], AF.Copy
                )
            qks.append(qk_t)

        # ---- phases B + C: software-pipelined over quads ----
        # iter i: scores(i) [PE repl + DVE STT], softhb(i-1) [ACT/GPS/PE],
        #         tail(i) [exp/dn/inv/wd].  C(b) emitted one iter after b ends.
        nq_total = nq
        h_tiles = {}
        scq_t = {}
        exp_t = {}
        inv_t = {}
        wd_t = {}
        hbar_t = {}
        cur_tiles = {}
        pend_c = None

        def emit_load(i):
            if i >= nq_total:
                return
            b, q = divmod(i, 8)
            if q == 0:
                cur_t = curpool.tile([128, 2, D], F32, tag="cur")
                nc.sync.dma_start(
                    out=cur_t[:, 0, :], in_=cur_r[128 * b : 128 * (b + 1), :]
                )
                nc.sync.dma_start(
                    out=cur_t[:, 1, :], in_=cur_i[128 * b : 128 * (b + 1), :]
                )
                cur_tiles[b] = cur_t
            h_t = hpool.tile([128, 4, E], F32R, tag="h")
            p0 = 16 * i
            nc.sync.dma_start(
                out=h_t[:, :, 0:D],
                in_=hist_r[p0 : p0 + 16].bitcast(F32R).rearrange(
                    "(gl j) s d -> (j s) gl d", j=4
                ),
            )
            nc.sync.dma_start(
                out=h_t[:, :, D:E],
                in_=hist_i[p0 : p0 + 16].bitcast(F32R).rearrange(
                    "(gl j) s d -> (j s) gl d", j=4
                ),
            )
            h_tiles[i] = h_t

        def emit_repl(i, gl):
            b, q = divmod(i, 8)
            lg = 4 * q + gl           # batch-local group id
            qkr = ps_qkr.tile([128, E], F32, tag="qkr")
            for h in range(2):
                nc.tensor.matmul(
                    qkr[:, 512 * h : 512 * (h + 1)],
                    lhsT=mask_t[:, lg, :],
                    rhs=qks[b][:, 512 * h : 512 * (h + 1)],
                    start=True,
                    stop=True,
                )
            return qkr

        def emit_stt(i, gl, qkr):
            b, q = divmod(i, 8)
            g = 32 * b + 4 * q + gl
            h_t = h_tiles[i]
            scq = scq_t[i]
            sink = sinkpool.tile([128, E], F16, tag="sink")
            nc.vector.scalar_tensor_tensor(
                out=sink[:],
                in0=h_t[:, gl, :].bitcast(F32),
                scalar=cr_t[:, g : g + 1],
                in1=qkr[:],
                op0=AX.mult,
                op1=AX.mult,
                accum_out=scq[:, gl : gl + 1],
            )

        def emit_hb(i, gl):
            b, q = divmod(i, 8)
            if gl == 0 and q == 0:
                hbar_new = hbpool.tile([128, E], F16, tag="hbar")
                hbar_t[b] = hbar_new
            hbar_b = hbar_t[b]
            h_t = h_tiles[i]
            inv4 = inv_t[i]
            wd = wd_t.pop((i, gl))
            hb = ps_hb.tile([4, E], F32, tag="hb")
            for h in range(2):
                nc.tensor.matmul(
                    hb[:, 512 * h : 512 * (h + 1)],
                    lhsT=wd[:],
                    rhs=h_t[:, gl, 512 * h : 512 * (h + 1)],
                    start=True,
                    stop=True,
                )
            hsb = hsbpool.tile([4, E], F16, tag="hsb")
            nc.scalar.activation(hsb[:], hb[:], AF.Copy,
                                 scale=inv4[:, gl : gl + 1])
            lp = 16 * q + 4 * gl
            nc.gpsimd.dma_start(out=hbar_b[lp : lp + 4, :], in_=hsb[:])
            if gl == 3:
                h_tiles.pop(i)
                inv_t.pop(i)

        def emit_iter(i):
            # interleaved PE stream: repl(i) g0,g1 | hb(i-1) g0,g1 | repl g2 |
            # hb g2 | repl g3 | hb g3 — keeps PE fed while DVE drains qkr slots
            do_s = i < nq_total
            do_h = i >= 1
            if do_s:
                scq = smpool.tile([128, 4], F32, tag="scores")
                scq_t[i] = scq
                qkr0 = emit_repl(i, 0)
                emit_stt(i, 0, qkr0)
                qkr1 = emit_repl(i, 1)
                emit_stt(i, 1, qkr1)
            if do_h:
                emit_hb(i - 1, 0)
                emit_hb(i - 1, 1)
            if do_s:
                qkr2 = emit_repl(i, 2)
                emit_stt(i, 2, qkr2)
            if do_h:
                emit_hb(i - 1, 2)
            if do_s:
                qkr3 = emit_repl(i, 3)
                emit_stt(i, 3, qkr3)
            if do_h:
                emit_hb(i - 1, 3)

        def emit_tail(i):
            scq = scq_t.pop(i)
            exp4 = smpool.tile([128, 4], F32, tag="exp")
            nc.scalar.activation(exp4[:], scq[:], AF.Exp)
            dn = ps_misc.tile([4, 4], F32, tag="misc")
            nc.tensor.matmul(dn[:], lhsT=m4_t[:], rhs=exp4[:],
                             start=True, stop=True)
            inv4 = smpool.tile([4, 4], F32, tag="invr")
            nc.vector.reciprocal(inv4[:], dn[:])
            inv_t[i] = inv4
            for gl in range(4):
                wd = wdpool.tile([128, 4], F32R, tag="wd")
                nc.gpsimd.tensor_scalar(
                    wd[:], m4_t[:], exp4[:, gl : gl + 1], None, AX.mult,
                )
                wd_t[(i, gl)] = wd

        def emit_c(b):
            hbar_b = hbar_t.pop(b)
            cur_t = cur_tiles.pop(b)
            ht0 = htpool.tile([128, 8, 128], F16, tag="hbarT")
            for c in range(8):
                tp = ps_misc.tile([128, 128], F16, tag="misc")
                nc.tensor.transpose(
                    tp[:], hbar_b[:, 128 * c : 128 * (c + 1)], id_t[:]
                )
                nc.scalar.activation(ht0[:, c, :], tp[:], AF.Copy)
            out_t = outpool.tile([128, D, 2], F32)
            for h2 in range(2):
                cps = ps_misc.tile([128, 512], F32, tag="misc")
                for c in range(8):
                    nc.tensor.matmul(
                        cps[:],
                        lhsT=ht0[:, c, :],
                        rhs=vw_t[:, c, 512 * h2 : 512 * (h2 + 1)],
                        start=(c == 0),
                        stop=(c == 7),
                    )
                nc.vector.scalar_tensor_tensor(
                    out=out_t[:, :, h2],
                    in0=cps[:],
                    scalar=0.1,
                    in1=cur_t[:, h2, :],
                    op0=AX.mult,
                    op1=AX.add,
                )
            nc.sync.dma_start(
                out=out[:].rearrange("(bb p) d two -> p bb d two", p=128)[:, b],
                in_=out_t[:],
            )

        for i in range(3):
            emit_load(i)
        for i in range(nq_total + 1):
            emit_load(i + 3)
            emit_iter(i)
            if pend_c is not None:
                emit_c(pend_c)
                pend_c = None
            if i >= 1 and i % 8 == 0:
                pend_c = i // 8 - 1
            if i < nq_total:
                emit_tail(i)
        emit_c(nb - 1)

    nc.compile()
    return nc


_CACHE: dict[int, bass.Bass] = {}


def get_nc(ppc: int) -> bass.Bass:
    if ppc not in _CACHE:
        _CACHE[ppc] = build(ppc)
    return _CACHE[ppc]


def _to_bf16(x: np.ndarray) -> np.ndarray:
    """Round f32 -> bf16 bit pattern, returned as uint16 view-compatible array."""
    import ml_dtypes
    return x.astype(ml_dtypes.bfloat16)


def make_const_inputs(ng: int):
    import ml_dtypes
    mask_v = np.zeros((128, 32, 128), np.float32)
    for lg in range(32):
        for j in range(4):
            mask_v[4 * lg + j, lg, 32 * j : 32 * (j + 1)] = 1.0
    m4_h = np.zeros((128, 4), np.float32)
    for j in range(4):
        m4_h[32 * j : 32 * (j + 1), j] = 1.0
    ones_h = np.ones((1, 128), np.float32)
    id_h = np.eye(128, dtype=np.float16)
    return (
        mask_v.astype(ml_dtypes.bfloat16),
        m4_h,
        ones_h.astype(ml_dtypes.bfloat16),
        id_h,
    )


def host_prep(hist_real, hist_imag, current_real, current_imag, confidence,
              qW, qb, kW, kb, vW, vb, ppc):
    """Shared host-side folding + per-core input maps."""
    f = lambda x: np.ascontiguousarray(np.asarray(x, dtype=np.float32))
    hist_real, hist_imag = f(hist_real), f(hist_imag)
    current_real, current_imag = f(current_real), f(current_imag)
    confidence = f(confidence)
    qW, qb, kW, kb, vW, vb = f(qW), f(qb), f(kW), f(kb), f(vW), f(vb)

    n_cores = (B * T) // ppc
    wqk_h = _to_bf16(np.ascontiguousarray(qW @ kW.T))    # [D2, E]
    bqk_h = _to_bf16((qb @ kW.T).reshape(1, E))          # [1, E]
    vw_h = vW.astype(np.float16)
    ng = ppc // 4
    mask_h, m4_h, ones1b_h, id_h = make_const_inputs(ng)

    hr = hist_real.reshape(B * T, S, D)
    hi = hist_imag.reshape(B * T, S, D)
    cr = current_real.reshape(B * T, D)
    ci = current_imag.reshape(B * T, D)
    cf = confidence.reshape(B * T)

    in_maps = []
    for c in range(n_cores):
        sl = slice(c * ppc, (c + 1) * ppc)
        cur2t_h = _to_bf16(np.ascontiguousarray(
            np.concatenate([cr[sl], ci[sl]], axis=1).T
        ))  # [D2, ppc]
        c4 = cf[sl].reshape(ng, 4).T * SCALE          # [4, ng]
        conf_rep_h = np.ascontiguousarray(np.repeat(c4, 32, axis=0))  # [128, ng]
        in_maps.append({
            "hist_real": hr[sl],
            "hist_imag": hi[sl],
            "cur_r": cr[sl],
            "cur_i": ci[sl],
            "cur2t": cur2t_h,
            "wqk": wqk_h,
            "bqk": bqk_h,
            "vw": vw_h,
            "conf_rep": conf_rep_h,
            "mask32": mask_h,
            "m4": m4_h,
            "ones1b": ones1b_h,
            "ident": id_h,
        })
    return in_maps


def kernel(hist_real, hist_imag, current_real, current_imag, confidence,
           qW, qb, kW, kb, vW, vb):
    ppc = PAIRS // N_CORES
    nc = get_nc(ppc)
    in_maps = host_prep(hist_real, hist_imag, current_real, current_imag,
                        confidence, qW, qb, kW, kb, vW, vb, ppc)
    res = run_bass_kernel_spmd(nc, in_maps, list(range(N_CORES))).results
    out = np.concatenate([res[c]["out"] for c in range(N_CORES)], axis=0)
    out = out.view(np.complex64)[..., 0].reshape(B, T, D)
    vb_f = np.asarray(vb, dtype=np.float32).reshape(E)
    out = out + 0.1 * (vb_f[:D] + 1j * vb_f[D:]).astype(np.complex64)
    return out
